# revision 4
# baseline (speedup 1.0000x reference)
"""Trainium2 Bass kernel for nn_AttentionMechanism (tanh-MLP attention), v3.

Math (per batch b):
  q[b]       = W_h_w @ h_t[b] + W_h_b + W_b                   (host, tiny)
  U[beta,s]  = sum_c W8[beta,c] V8[c,s]     (PE, fp8e4 DoubleRow: K=256 in one
               pass, 512 rows per instr; W8 = fp8(W_w*64), V8 = fp8(V))
  T = tanh(U/64 + q)                        (ACT, out bf16)
  E[s]       = sum_beta bw[beta] T[beta,s]  (PE, bf16, K=128 x4 chunks)
  w[s] = exp(E)     (E row copied psum->SBUF by DVE, reshape-DMA to [128,8]
                     (s = p*8 + j), ACT exp per 4-batch group on [128,32])
  P[c], SE   = sum_s w[s] * V2[s, c|ones]   (PE, bf16, w-column stationary)
  C[b,0,c]   = sum_cores P / sum_cores SE   (host, tiny)

Sharding: 4-way positions x 2-way batch halves; 32 batches x 1024 positions
per core.  ACT (tanh, 4x[128,1024]/batch ~ 147us) and PE (10248 rows/batch
~ 137us) are co-critical; matmul wall rate is 0.4167ns/row regardless of
dtype, so fp8 DoubleRow only pays via K=256-per-instruction on the U stage.
"""

import sys
from contextlib import ExitStack

import numpy as np

if "/opt/trn_rl_repo" not in sys.path:
    sys.path.insert(0, "/opt/trn_rl_repo")

import ml_dtypes

BF16 = ml_dtypes.bfloat16
F8 = ml_dtypes.float8_e4m3

HP, WP, C_DIM, B = 64, 64, 256, 64
BETA, HIDDEN = 512, 512
NCORES = 8
N_HPQ = 4                      # position shards
N_BH = 2                       # batch shards
B_CORE = B // N_BH             # 32 batches per core
S_CORE = (HP // N_HPQ) * WP    # 1024 positions per core
NCH = S_CORE // 128            # 8 interleaved s-chunks for the P stage
SW = 64.0                      # W_w fp8 pre-scale
EGROUP = 4                     # batches per exp instruction

_NC_CACHE = {}


def _build_nc():
    import concourse.bass as bass
    import concourse.bacc as bacc
    import concourse.tile as tile
    import concourse.mybir as mybir
    from concourse.mybir import dt

    AF = mybir.ActivationFunctionType
    PM = mybir.MatmulPerfMode.DoubleRow
    f32, bf16, f8 = dt.float32, dt.bfloat16, dt.float8e4

    nc = bacc.Bacc("TRN2", target_bir_lowering=False, debug=False,
                   num_devices=NCORES)

    # [part c%128, b, c-chunk, s] fp8 for the U matmul rhs
    v8_d = nc.dram_tensor("v8", [128, B_CORE, 2, S_CORE], f8,
                          kind="ExternalInput")
    # [part s//8, b, s%8, c(+ones)] bf16 for the P matmul rhs
    v2_d = nc.dram_tensor("v2", [128, B_CORE, NCH, C_DIM + 1], bf16,
                          kind="ExternalInput")
    # W8 = fp8(W_w*64)^T: [part c%128, c-chunk, beta]
    wt8_d = nc.dram_tensor("wt8", [128, 2, BETA], f8, kind="ExternalInput")
    # tanh biases: [part, m*B_CORE + b]
    qs_d = nc.dram_tensor("qs", [128, 4 * B_CORE], f32, kind="ExternalInput")
    # bw replicated over output cols: [part b%128, m-chunk, po]
    bw16_d = nc.dram_tensor("bw16", [128, 4, 128], bf16,
                            kind="ExternalInput")
    # per-batch [P[0:256], SE] row
    pse_d = nc.dram_tensor("pse", [B_CORE, C_DIM + 1], f32,
                           kind="ExternalOutput")

    with tile.TileContext(nc) as tc, ExitStack() as ctx:
        cpool = ctx.enter_context(tc.tile_pool(name="const", bufs=1))
        v8pool = ctx.enter_context(tc.tile_pool(name="v8p", bufs=8))
        v2pool = ctx.enter_context(tc.tile_pool(name="v2p", bufs=8))
        tpool = ctx.enter_context(tc.tile_pool(name="tp", bufs=8))
        etpool = ctx.enter_context(tc.tile_pool(name="etp", bufs=2))
        espool = ctx.enter_context(tc.tile_pool(name="esp", bufs=2))
        wpool = ctx.enter_context(tc.tile_pool(name="wp", bufs=2))
        opool = ctx.enter_context(tc.tile_pool(name="op", bufs=2))
        upsum = ctx.enter_context(tc.tile_pool(name="ups", bufs=2,
                                               space="PSUM"))
        epsum = ctx.enter_context(tc.tile_pool(name="eps", bufs=1,
                                               space="PSUM"))
        ppsum = ctx.enter_context(tc.tile_pool(name="pps", bufs=2,
                                               space="PSUM"))

        # ---- constants ----
        wt8 = cpool.tile([128, 2, BETA], f8, tag="wt8")
        nc.sync.dma_start(wt8, wt8_d[:])
        qs = cpool.tile([128, 4 * B_CORE], f32, tag="qs")
        nc.sync.dma_start(qs, qs_d[:])
        bw16 = cpool.tile([128, 4, 128], bf16, tag="bw16")
        nc.sync.dma_start(bw16, bw16_d[:])

        # ---- per-batch V tiles (streamed) ----
        v8t = [None] * B_CORE
        v2t = [None] * B_CORE
        for b in range(B_CORE):
            v8t[b] = v8pool.tile([128, 2, S_CORE], f8, tag="v8",
                                 name=f"v8b{b}")
            # sliced so the first U matmul waits on a 64KB piece, not 256KB
            for k in range(2):
                for sh in range(2):
                    sl = slice(sh * (S_CORE // 2), (sh + 1) * (S_CORE // 2))
                    nc.sync.dma_start(v8t[b][:, k, sl], v8_d[:, b, k, sl])
            v2t[b] = v2pool.tile([128, NCH, C_DIM + 1], bf16, tag="v2",
                                 name=f"v2b{b}")
            for j2 in range(2):
                nc.sync.dma_start(
                    v2t[b][:, j2 * (NCH // 2):(j2 + 1) * (NCH // 2), :],
                    v2_d[:, b, j2 * (NCH // 2):(j2 + 1) * (NCH // 2), :])

        t16 = [None] * B_CORE         # 4 tanh tiles per batch, bf16
        et = [None] * (B_CORE // EGROUP)
        inv64 = 1.0 / SW
        NSH = S_CORE // 512

        def do_U(b):
            t16[b] = [tpool.tile([128, S_CORE], bf16, tag=f"t16m{m}",
                                 name=f"t16b{b}m{m}") for m in range(4)]
            for m in range(4):
                u = upsum.tile([128, S_CORE], f32, tag="u", name=f"u{b}m{m}")
                for sh in range(NSH):
                    sl = slice(sh * 512, (sh + 1) * 512)
                    nc.tensor.matmul(
                        u[:, sl], wt8[:, :, m * 128:(m + 1) * 128],
                        v8t[b][:, :, sl],
                        start=True, stop=True, perf_mode=PM)
                nc.scalar.activation(
                    t16[b][m], u, AF.Tanh,
                    bias=qs[:, m * B_CORE + b:m * B_CORE + b + 1],
                    scale=inv64)

        def do_E(b):
            e = epsum.tile([128, S_CORE], f32, tag="e", name=f"e{b}")
            for sh in range(NSH):
                sl = slice(sh * 512, (sh + 1) * 512)
                for m in range(4):
                    nc.tensor.matmul(
                        e[:, sl], bw16[:, m], t16[b][m][:, sl],
                        start=(m == 0), stop=(m == 3))
            g, gi = b // EGROUP, b % EGROUP
            if gi == 0:
                et[g] = etpool.tile([128, EGROUP * NCH], f32, tag="et",
                                    name=f"et{g}")
            # E row psum -> SBUF (DVE), then reshape-DMA (s = p*8 + j)
            # -> [128 p, 8 j]; both DMA sides are contiguous runs
            es = espool.tile([1, S_CORE], f32, tag="es", name=f"es{b}")
            nc.vector.tensor_copy(es, e[0:1, :])
            nc.sync.dma_start(
                et[g][:, gi * NCH:(gi + 1) * NCH],
                es.rearrange("p (q j) -> p q j", j=NCH))
            t16[b] = None

        wg_t = [None] * (B_CORE // EGROUP)

        def do_exp(g):
            wg_t[g] = wpool.tile([128, EGROUP * NCH], bf16, tag="w",
                                 name=f"w{g}")
            nc.scalar.activation(wg_t[g], et[g], AF.Exp)

        def do_P1(b):
            g, gi = b // EGROUP, b % EGROUP
            p = ppsum.tile([1, C_DIM + 1], f32, tag="p", name=f"p{b}")
            for j in range(NCH):
                nc.tensor.matmul(
                    p, wg_t[g][:, gi * NCH + j:gi * NCH + j + 1],
                    v2t[b][:, j, :],
                    start=(j == 0), stop=(j == NCH - 1))
            po = opool.tile([1, C_DIM + 1], f32, tag="po", name=f"po{b}")
            nc.vector.tensor_copy(po, p)
            nc.sync.dma_start(pse_d[b:b + 1, :], po)

        # software pipeline: E lags U by one batch so the PE queue never
        # blocks on the current batch's tanh; P spread one batch per iter
        # so the PE queue has no burst at exp-group boundaries
        for b in range(B_CORE):
            do_U(b)
            if b > 0:
                do_E(b - 1)
            if b % EGROUP == 0 and b >= EGROUP:
                do_exp(b // EGROUP - 1)
            if b >= EGROUP:
                do_P1(b - EGROUP)
        do_E(B_CORE - 1)
        do_exp(B_CORE // EGROUP - 1)
        for b in range(B_CORE - EGROUP, B_CORE):
            do_P1(b)

    nc.compile()
    return nc


def _get_nc():
    if "nc" not in _NC_CACHE:
        _NC_CACHE["nc"] = _build_nc()
    return _NC_CACHE["nc"]


def _host_smalls(h_t, W_h_w, W_h_b, W_w, W_b, beta_w):
    q = h_t[:, 0, :].astype(np.float64) @ W_h_w.T.astype(np.float64) \
        + W_h_b + W_b                                  # [b, beta]
    qs3 = q.T.reshape(4, 128, B).transpose(1, 0, 2)    # [128, 4, 64]
    qs_h = [np.ascontiguousarray(
        qs3[:, :, bh * B_CORE:(bh + 1) * B_CORE].reshape(128, 4 * B_CORE)
    ).astype(np.float32) for bh in range(N_BH)]

    # W8 = fp8(W_w*64)^T: wt8[p, k, m] = W8[m, k*128+p]
    w8 = np.asarray(W_w.astype(np.float32) * SW, F8)   # [beta, c]
    wt8 = np.ascontiguousarray(
        w8.reshape(BETA, 2, 128).transpose(2, 1, 0))

    # bw replicated: bw16[p, m, po] = bw[m*128+p]
    bwv = beta_w[0].astype(np.float32).reshape(4, 128)  # [m, p]
    bw16 = np.ascontiguousarray(
        np.repeat(bwv.T[:, :, None], 128, axis=2)).astype(BF16)
    return qs_h, wt8, bw16


_PROFILE = False
_LAST_PERF = {}


def kernel(**inputs):
    from concourse.bass_utils import run_bass_kernel_spmd

    V = np.asarray(inputs["V"], dtype=np.float32)
    h_t = np.asarray(inputs["h_t"], dtype=np.float32)
    W_h_w = np.asarray(inputs["W_h_w"], dtype=np.float32)
    W_h_b = np.asarray(inputs["W_h_b"], dtype=np.float32)
    W_w = np.asarray(inputs["W_w"], dtype=np.float32)
    W_b = np.asarray(inputs["W_b"], dtype=np.float32)
    beta_w = np.asarray(inputs["beta_w"], dtype=np.float32)
    beta_b = np.asarray(inputs["beta_b"], dtype=np.float32)

    qs_h, wt8, bw16 = _host_smalls(h_t, W_h_w, W_h_b, W_w, W_b, beta_w)

    rows = HP // N_HPQ
    in_maps = []
    core_meta = []
    for kq in range(N_HPQ):
        # [s, c, b] slab for this position shard
        Vq = V[kq * rows:(kq + 1) * rows].reshape(S_CORE, C_DIM, B)
        for bh in range(N_BH):
            Vqb = Vq[:, :, bh * B_CORE:(bh + 1) * B_CORE]   # [s, c, b]
            # v8[p, b, k, s] = V[c=k*128+p, s, b]
            v8 = np.ascontiguousarray(
                Vqb.reshape(S_CORE, 2, 128, B_CORE)
                .transpose(2, 3, 1, 0)).astype(F8)
            # v2[p, b, j, c] = V[c, s=p*8+j, b]; col 256 = 1
            v2 = np.empty((128, B_CORE, NCH, C_DIM + 1), BF16)
            v2[:, :, :, :C_DIM] = (
                Vqb.reshape(128, NCH, C_DIM, B_CORE)
                .transpose(0, 3, 1, 2)).astype(BF16)
            v2[:, :, :, C_DIM] = np.asarray(1.0, BF16)
            in_maps.append({"v8": v8, "v2": v2, "wt8": wt8,
                            "qs": qs_h[bh], "bw16": bw16})
            core_meta.append(bh)

    nc = _get_nc()
    res = run_bass_kernel_spmd(nc, in_maps, core_ids=list(range(NCORES)),
                               trace=_PROFILE)
    if _PROFILE:
        _LAST_PERF["exec_time_ns"] = res.exec_time_ns
        _LAST_PERF["trace"] = res.instructions_and_trace
    P = np.zeros((B, C_DIM), np.float64)
    SE = np.zeros((B,), np.float64)
    for bh, r in zip(core_meta, res.results):
        sl = slice(bh * B_CORE, (bh + 1) * B_CORE)
        pse = r["pse"].astype(np.float64)
        P[sl] += pse[:, :C_DIM]
        SE[sl] += pse[:, C_DIM]
    C = (P / SE[:, None]).reshape(B, 1, C_DIM)
    return C.astype(np.float32)


# revision 6
# speedup vs baseline: 1.1799x; 1.1799x over previous
"""Trainium2 Bass kernel for nn_AttentionMechanism (tanh-MLP attention), v3.

Math (per batch b):
  q[b]       = W_h_w @ h_t[b] + W_h_b + W_b                   (host, tiny)
  U[beta,s]  = sum_c W8[beta,c] V8[c,s]     (PE, fp8e4 DoubleRow: K=256 in one
               pass, 512 rows per instr; W8 = fp8(W_w*64), V8 = fp8(V))
  T = tanh(U/64 + q)                        (ACT, out bf16)
  E[s]       = sum_beta bw[beta] T[beta,s]  (PE, bf16, K=128 x4 chunks)
  w[s] = exp(E)     (E row copied psum->SBUF by DVE, reshape-DMA to [128,8]
                     (s = p*8 + j), ACT exp per 4-batch group on [128,32])
  P[c], SE   = sum_s w[s] * V2[s, c|ones]   (PE, bf16, w-column stationary)
  C[b,0,c]   = sum_cores P / sum_cores SE   (host, tiny)

Sharding: 4-way positions x 2-way batch halves; 32 batches x 1024 positions
per core.  ACT (tanh, 4x[128,1024]/batch ~ 147us) and PE (10248 rows/batch
~ 137us) are co-critical; matmul wall rate is 0.4167ns/row regardless of
dtype, so fp8 DoubleRow only pays via K=256-per-instruction on the U stage.
"""

import sys
from contextlib import ExitStack

import numpy as np

if "/opt/trn_rl_repo" not in sys.path:
    sys.path.insert(0, "/opt/trn_rl_repo")

import ml_dtypes

BF16 = ml_dtypes.bfloat16
F8 = ml_dtypes.float8_e4m3

HP, WP, C_DIM, B = 64, 64, 256, 64
BETA, HIDDEN = 512, 512
NCORES = 8
N_HPQ = 4                      # position shards
N_BH = 2                       # batch shards
B_CORE = B // N_BH             # 32 batches per core
S_CORE = (HP // N_HPQ) * WP    # 1024 positions per core
NCH = S_CORE // 128            # 8 interleaved s-chunks for the P stage
SW = 64.0                      # W_w fp8 pre-scale
EGROUP = 4                     # batches per exp instruction

_NC_CACHE = {}


def _build_nc():
    import concourse.bass as bass
    import concourse.bacc as bacc
    import concourse.tile as tile
    import concourse.mybir as mybir
    from concourse.mybir import dt

    AF = mybir.ActivationFunctionType
    PM = mybir.MatmulPerfMode.DoubleRow
    f32, bf16, f8 = dt.float32, dt.bfloat16, dt.float8e4

    nc = bacc.Bacc("TRN2", target_bir_lowering=False, debug=False,
                   num_devices=NCORES)

    # [part c%128, b, c-chunk, s] fp8 for the U matmul rhs
    v8_d = nc.dram_tensor("v8", [128, B_CORE, 2, S_CORE], f8,
                          kind="ExternalInput")
    # [part s//8, b, s%8, c(+ones)] bf16 for the P matmul rhs
    v2_d = nc.dram_tensor("v2", [128, B_CORE, NCH, C_DIM + 1], bf16,
                          kind="ExternalInput")
    # W8 = fp8(W_w*64)^T: [part c%128, c-chunk, beta]
    wt8_d = nc.dram_tensor("wt8", [128, 2, BETA], f8, kind="ExternalInput")
    # tanh biases: [part, m*B_CORE + b]
    qs_d = nc.dram_tensor("qs", [128, 4 * B_CORE], f32, kind="ExternalInput")
    # bw replicated over output cols: [part b%128, m-chunk, po]
    bw16_d = nc.dram_tensor("bw16", [128, 4, 128], bf16,
                            kind="ExternalInput")
    # per-batch [P[0:256], SE] row
    pse_d = nc.dram_tensor("pse", [B_CORE, C_DIM + 1], f32,
                           kind="ExternalOutput")

    with tile.TileContext(nc) as tc, ExitStack() as ctx:
        cpool = ctx.enter_context(tc.tile_pool(name="const", bufs=1))
        v8pool = ctx.enter_context(tc.tile_pool(name="v8p", bufs=8))
        v2pool = ctx.enter_context(tc.tile_pool(name="v2p", bufs=8))
        tpool = ctx.enter_context(tc.tile_pool(name="tp", bufs=8))
        etpool = ctx.enter_context(tc.tile_pool(name="etp", bufs=2))
        espool = ctx.enter_context(tc.tile_pool(name="esp", bufs=2))
        wpool = ctx.enter_context(tc.tile_pool(name="wp", bufs=2))
        opool = ctx.enter_context(tc.tile_pool(name="op", bufs=2))
        upsum = ctx.enter_context(tc.tile_pool(name="ups", bufs=2,
                                               space="PSUM"))
        epsum = ctx.enter_context(tc.tile_pool(name="eps", bufs=1,
                                               space="PSUM"))
        ppsum = ctx.enter_context(tc.tile_pool(name="pps", bufs=2,
                                               space="PSUM"))

        # ---- constants ----
        wt8 = cpool.tile([128, 2, BETA], f8, tag="wt8")
        nc.sync.dma_start(wt8, wt8_d[:])
        qs = cpool.tile([128, 4 * B_CORE], f32, tag="qs")
        nc.sync.dma_start(qs, qs_d[:])
        bw16 = cpool.tile([128, 4, 128], bf16, tag="bw16")
        nc.sync.dma_start(bw16, bw16_d[:])

        # ---- per-batch V tiles (streamed) ----
        v8t = [None] * B_CORE
        v2t = [None] * B_CORE
        for b in range(B_CORE):
            v8t[b] = v8pool.tile([128, 2, S_CORE], f8, tag="v8",
                                 name=f"v8b{b}")
            if b == 0:
                # sliced so the first U matmul waits on 64KB, not 256KB
                for k in range(2):
                    for sh in range(2):
                        sl = slice(sh * (S_CORE // 2),
                                   (sh + 1) * (S_CORE // 2))
                        nc.sync.dma_start(v8t[b][:, k, sl],
                                          v8_d[:, b, k, sl])
            else:
                nc.sync.dma_start(v8t[b], v8_d[:, b])
        v2pair = [None] * (B_CORE // 2)
        for h in range(B_CORE // 2):
            v2pair[h] = v2pool.tile([128, 2, NCH, C_DIM + 1], bf16,
                                    tag="v2", name=f"v2h{h}")
            nc.sync.dma_start(v2pair[h], v2_d[:, 2 * h:2 * h + 2])
            v2t[2 * h] = v2pair[h][:, 0]
            v2t[2 * h + 1] = v2pair[h][:, 1]

        t16 = [None] * B_CORE         # 4 tanh tiles per batch, bf16
        et = [None] * (B_CORE // EGROUP)
        es4 = [None] * (B_CORE // EGROUP)
        po4 = [None] * (B_CORE // EGROUP)
        inv64 = 1.0 / SW
        NSH = S_CORE // 512

        def do_U(b):
            t16[b] = [tpool.tile([128, S_CORE], bf16, tag=f"t16m{m}",
                                 name=f"t16b{b}m{m}") for m in range(4)]
            for m in range(4):
                u = upsum.tile([128, S_CORE], f32, tag="u", name=f"u{b}m{m}")
                for sh in range(NSH):
                    sl = slice(sh * 512, (sh + 1) * 512)
                    nc.tensor.matmul(
                        u[:, sl], wt8[:, :, m * 128:(m + 1) * 128],
                        v8t[b][:, :, sl],
                        start=True, stop=True, perf_mode=PM)
                nc.scalar.activation(
                    t16[b][m], u, AF.Tanh,
                    bias=qs[:, m * B_CORE + b:m * B_CORE + b + 1],
                    scale=inv64)

        def do_E(b):
            e = epsum.tile([128, S_CORE], f32, tag="e", name=f"e{b}")
            for sh in range(NSH):
                sl = slice(sh * 512, (sh + 1) * 512)
                for m in range(4):
                    nc.tensor.matmul(
                        e[:, sl], bw16[:, m], t16[b][m][:, sl],
                        start=(m == 0), stop=(m == 3))
            g, gi = b // EGROUP, b % EGROUP
            if gi == 0:
                et[g] = etpool.tile([128, EGROUP * NCH], f32, tag="et",
                                    name=f"et{g}")
                es4[g] = espool.tile([1, EGROUP * S_CORE], f32, tag="es",
                                     name=f"es{g}")
            # E row psum -> SBUF; DVE writes the group tile pre-interleaved
            # as (q, g, j) so one contiguous DMA per group lands it as
            # et[p=q, (g j)] (the s = p*8 + j chunking for the P stage)
            nc.vector.tensor_copy(
                es4[g].rearrange("o (q g j) -> o q g j",
                                 g=EGROUP, j=NCH)[:, :, gi, :],
                e[0:1, :])
            if gi == EGROUP - 1:
                nc.sync.dma_start(
                    et[g], es4[g].rearrange("o (q x) -> o q x", q=128))
            t16[b] = None

        wg_t = [None] * (B_CORE // EGROUP)

        def do_exp(g):
            wg_t[g] = wpool.tile([128, EGROUP * NCH], bf16, tag="w",
                                 name=f"w{g}")
            nc.scalar.activation(wg_t[g], et[g], AF.Exp)

        def do_P1(b):
            g, gi = b // EGROUP, b % EGROUP
            p = ppsum.tile([1, C_DIM + 1], f32, tag="p", name=f"p{b}")
            for j in range(NCH):
                nc.tensor.matmul(
                    p, wg_t[g][:, gi * NCH + j:gi * NCH + j + 1],
                    v2t[b][:, j, :],
                    start=(j == 0), stop=(j == NCH - 1))
            if gi == 0:
                po4[g] = opool.tile([1, EGROUP * (C_DIM + 1)], f32,
                                    tag="po", name=f"po{g}")
            nc.vector.tensor_copy(
                po4[g][:, gi * (C_DIM + 1):(gi + 1) * (C_DIM + 1)], p)
            if gi == EGROUP - 1:
                nc.sync.dma_start(
                    pse_d[g * EGROUP:(g + 1) * EGROUP, :],
                    po4[g].rearrange("o (g c) -> o g c", g=EGROUP))

        # software pipeline: E lags U by one batch so the PE queue never
        # blocks on the current batch's tanh; P spread one batch per iter
        # so the PE queue has no burst at exp-group boundaries
        for b in range(B_CORE):
            do_U(b)
            if b > 0:
                do_E(b - 1)
            if b % EGROUP == 0 and b >= EGROUP:
                do_exp(b // EGROUP - 1)
            if b >= EGROUP:
                do_P1(b - EGROUP)
        do_E(B_CORE - 1)
        do_exp(B_CORE // EGROUP - 1)
        for b in range(B_CORE - EGROUP, B_CORE):
            do_P1(b)

    nc.compile()
    return nc


def _get_nc():
    if "nc" not in _NC_CACHE:
        _NC_CACHE["nc"] = _build_nc()
    return _NC_CACHE["nc"]


def _host_smalls(h_t, W_h_w, W_h_b, W_w, W_b, beta_w):
    q = h_t[:, 0, :].astype(np.float64) @ W_h_w.T.astype(np.float64) \
        + W_h_b + W_b                                  # [b, beta]
    qs3 = q.T.reshape(4, 128, B).transpose(1, 0, 2)    # [128, 4, 64]
    qs_h = [np.ascontiguousarray(
        qs3[:, :, bh * B_CORE:(bh + 1) * B_CORE].reshape(128, 4 * B_CORE)
    ).astype(np.float32) for bh in range(N_BH)]

    # W8 = fp8(W_w*64)^T: wt8[p, k, m] = W8[m, k*128+p]
    w8 = np.asarray(W_w.astype(np.float32) * SW, F8)   # [beta, c]
    wt8 = np.ascontiguousarray(
        w8.reshape(BETA, 2, 128).transpose(2, 1, 0))

    # bw replicated: bw16[p, m, po] = bw[m*128+p]
    bwv = beta_w[0].astype(np.float32).reshape(4, 128)  # [m, p]
    bw16 = np.ascontiguousarray(
        np.repeat(bwv.T[:, :, None], 128, axis=2)).astype(BF16)
    return qs_h, wt8, bw16


_PROFILE = False
_LAST_PERF = {}


def kernel(**inputs):
    from concourse.bass_utils import run_bass_kernel_spmd

    V = np.asarray(inputs["V"], dtype=np.float32)
    h_t = np.asarray(inputs["h_t"], dtype=np.float32)
    W_h_w = np.asarray(inputs["W_h_w"], dtype=np.float32)
    W_h_b = np.asarray(inputs["W_h_b"], dtype=np.float32)
    W_w = np.asarray(inputs["W_w"], dtype=np.float32)
    W_b = np.asarray(inputs["W_b"], dtype=np.float32)
    beta_w = np.asarray(inputs["beta_w"], dtype=np.float32)
    beta_b = np.asarray(inputs["beta_b"], dtype=np.float32)

    qs_h, wt8, bw16 = _host_smalls(h_t, W_h_w, W_h_b, W_w, W_b, beta_w)

    rows = HP // N_HPQ
    in_maps = []
    core_meta = []
    for kq in range(N_HPQ):
        # [s, c, b] slab for this position shard
        Vq = V[kq * rows:(kq + 1) * rows].reshape(S_CORE, C_DIM, B)
        for bh in range(N_BH):
            Vqb = Vq[:, :, bh * B_CORE:(bh + 1) * B_CORE]   # [s, c, b]
            # v8[p, b, k, s] = V[c=k*128+p, s, b]
            v8 = np.ascontiguousarray(
                Vqb.reshape(S_CORE, 2, 128, B_CORE)
                .transpose(2, 3, 1, 0)).astype(F8)
            # v2[p, b, j, c] = V[c, s=p*8+j, b]; col 256 = 1
            v2 = np.empty((128, B_CORE, NCH, C_DIM + 1), BF16)
            v2[:, :, :, :C_DIM] = (
                Vqb.reshape(128, NCH, C_DIM, B_CORE)
                .transpose(0, 3, 1, 2)).astype(BF16)
            v2[:, :, :, C_DIM] = np.asarray(1.0, BF16)
            in_maps.append({"v8": v8, "v2": v2, "wt8": wt8,
                            "qs": qs_h[bh], "bw16": bw16})
            core_meta.append(bh)

    nc = _get_nc()
    res = run_bass_kernel_spmd(nc, in_maps, core_ids=list(range(NCORES)),
                               trace=_PROFILE)
    if _PROFILE:
        _LAST_PERF["exec_time_ns"] = res.exec_time_ns
        _LAST_PERF["trace"] = res.instructions_and_trace
    P = np.zeros((B, C_DIM), np.float64)
    SE = np.zeros((B,), np.float64)
    for bh, r in zip(core_meta, res.results):
        sl = slice(bh * B_CORE, (bh + 1) * B_CORE)
        pse = r["pse"].astype(np.float64)
        P[sl] += pse[:, :C_DIM]
        SE[sl] += pse[:, C_DIM]
    C = (P / SE[:, None]).reshape(B, 1, C_DIM)
    return C.astype(np.float32)


# revision 8
# speedup vs baseline: 1.2135x; 1.0285x over previous
"""Trainium2 Bass kernel for nn_AttentionMechanism (tanh-MLP attention).

Math (per batch b):
  q[:, b]   = W_h_w @ h_t[b] + W_h_b + W_b                  (host, tiny)
  U[beta,s,b] = sum_c W_w[beta,c] V[c,s,b]                   (PE)
  T = tanh(U + q)     (q folded in as the ACT per-partition bias)
  E[s,b]    = sum_beta bw[beta] T[beta,s,b]                  (PE, output replicated over partitions)
  w = exp(E)          (no max-subtraction needed: |E| <= ||bw||_1 ~ 8)
  P[c,b]    = sum_s w[s,b] V[c,s,b]                          (DVE affine_mul_reduce)
  SE[b]     = sum_s w[s,b]                                   (DVE tensor_scalar accum)
  C[b,0,c]  = sum_cores P / sum_cores SE                     (host, tiny)

Sharding: 2D - 4-way over positions (hp quarters) x 2-way over batch
halves.  Each core gets s=1024 positions x 32 batches (32MB of V);
softmax combined on host over the 4 position-shards of each batch half.
The s=1024 per (core, batch) makes every ACT instruction FD>=1024,
amortizing the per-instruction overhead that bounded the 1D version.

Host pre-lays V out per-core as [c, b, s] bf16 (the sharding-prep copy),
so the device DMA reads contiguous runs at full HBM bandwidth, every
matmul rhs is s-contiguous (full PE rate), and the DVE P stage is a
single fused multiply-accumulate per (c-chunk, batch).
"""

import sys
from contextlib import ExitStack

import numpy as np

if "/opt/trn_rl_repo" not in sys.path:
    sys.path.insert(0, "/opt/trn_rl_repo")

import ml_dtypes

BF16 = ml_dtypes.bfloat16

HP, WP, C_DIM, B = 64, 64, 256, 64
BETA, HIDDEN = 512, 512
NCORES = 8
N_HPQ = 4                      # position shards
N_BH = 2                       # batch shards
B_CORE = B // N_BH             # 32 batches per core
S_CORE = (HP // N_HPQ) * WP    # 1024 positions per core
B_OCT = 2                      # batches per DMA tile

_NC_CACHE = {}


def _build_nc(s_core=S_CORE):
    import concourse.bass as bass
    import concourse.bacc as bacc
    import concourse.tile as tile
    import concourse.mybir as mybir
    from concourse.mybir import dt

    AF = mybir.ActivationFunctionType
    ALU = mybir.AluOpType
    f32, bf16 = dt.float32, dt.bfloat16

    n_oct = B_CORE // B_OCT
    n_sh = s_core // 512           # matmul N=512 tiles per batch

    nc = bacc.Bacc("TRN2", target_bir_lowering=False, debug=False,
                   num_devices=NCORES)

    v_d = nc.dram_tensor("v", [C_DIM, B_CORE, s_core], bf16,
                         kind="ExternalInput")
    wt_d = nc.dram_tensor("wt", [128, 2 * BETA], bf16, kind="ExternalInput")
    qs_d = nc.dram_tensor("qs", [128, 4 * B_CORE], f32, kind="ExternalInput")
    bwr_d = nc.dram_tensor("bwr", [128, BETA], bf16, kind="ExternalInput")
    p_d = nc.dram_tensor("p_out", [2, 128, B_CORE], f32, kind="ExternalOutput")
    se_d = nc.dram_tensor("se_out", [1, B_CORE], f32, kind="ExternalOutput")

    with tile.TileContext(nc) as tc, ExitStack() as ctx:
        cpool = ctx.enter_context(tc.tile_pool(name="const", bufs=1))
        vpool = ctx.enter_context(tc.tile_pool(name="vp", bufs=1))
        tpool = ctx.enter_context(tc.tile_pool(name="tp", bufs=5))
        wpool = ctx.enter_context(tc.tile_pool(name="wp", bufs=2))
        ppool = ctx.enter_context(tc.tile_pool(name="pp", bufs=2))
        apool = ctx.enter_context(tc.tile_pool(name="ap", bufs=1))
        psum = ctx.enter_context(tc.tile_pool(name="ps", bufs=4, space="PSUM"))

        # ---- constants ----
        wt_sb = cpool.tile([128, 2 * BETA], bf16, tag="wt")
        nc.sync.dma_start(wt_sb, wt_d[:])
        qs_sb = cpool.tile([128, 4 * B_CORE], f32, tag="qs")
        nc.sync.dma_start(qs_sb, qs_d[:])
        bwr_sb = cpool.tile([128, BETA], bf16, tag="bwr")
        nc.sync.dma_start(bwr_sb, bwr_d[:])

        # ---- V tiles resident; first pair split to single-b tiles so the
        # first matmuls wait on 512KB instead of 2MB ----
        vb = [[None, None] for _ in range(B_CORE)]
        for b in range(B_OCT):
            for k in range(2):
                t = vpool.tile([128, s_core], bf16, tag=f"vs{k}b{b}",
                               name=f"vs{k}b{b}")
                nc.sync.dma_start(t, v_d[k * 128:(k + 1) * 128, b, :])
                vb[b][k] = t
        for o in range(1, n_oct):
            for k in range(2):
                t = vpool.tile([128, B_OCT * s_core], bf16, tag=f"v{k}o{o}",
                               name=f"v{k}o{o}")
                nc.sync.dma_start(
                    t, v_d[k * 128:(k + 1) * 128, o * B_OCT:(o + 1) * B_OCT, :])
                view = t.rearrange("p (b s) -> p b s", s=s_core)
                for h in range(B_OCT):
                    vb[o * B_OCT + h][k] = view[:, h, :]

        # ---- output accumulators ----
        p_fin = [apool.tile([128, B_CORE], f32, tag=f"pfin{k}",
                            name=f"pfin{k}") for k in range(2)]
        se_fin = apool.tile([128, B_CORE], f32, tag="sefin")

        for b in range(B_CORE):
            t_tiles = []
            for m in range(4):
                u = psum.tile([128, s_core], f32, tag="acc", name="u")
                for kp in range(2):
                    for sh in range(n_sh):
                        nc.tensor.matmul(
                            u[:, sh * 512:(sh + 1) * 512],
                            wt_sb[:, kp * BETA + m * 128:
                                  kp * BETA + (m + 1) * 128],
                            vb[b][kp][:, sh * 512:(sh + 1) * 512],
                            start=(kp == 0), stop=(kp == 1))
                t_m = tpool.tile([128, s_core], bf16, tag="t", name="t_m")
                nc.scalar.activation(
                    t_m, u, AF.Tanh,
                    bias=qs_sb[:, m * B_CORE + b:m * B_CORE + b + 1])
                t_tiles.append(t_m)

            e_rep = psum.tile([128, s_core], f32, tag="acc", name="e_rep")
            for m in range(4):
                for sh in range(n_sh):
                    nc.tensor.matmul(
                        e_rep[:, sh * 512:(sh + 1) * 512],
                        bwr_sb[:, m * 128:(m + 1) * 128],
                        t_tiles[m][:, sh * 512:(sh + 1) * 512],
                        start=(m == 0), stop=(m == 3))
            w_rep = wpool.tile([128, s_core], bf16, tag="w", name="w_rep")
            nc.scalar.activation(w_rep, e_rep, AF.Exp)

            for k in range(2):
                prod = ppool.tile([128, s_core], bf16, tag="prod",
                                  name="prod")
                nc.vector.affine_mul_reduce(
                    out=prod, accum_out=p_fin[k][:, b:b + 1],
                    in0=vb[b][k], in1=w_rep,
                    scale=1.0, bias=0.0)
            sescr = ppool.tile([128, s_core], bf16, tag="sescr",
                               name="sescr")
            nc.vector.tensor_scalar(
                sescr, w_rep, 1.0, None, op0=ALU.mult, op1=ALU.add,
                accum_out=se_fin[:, b:b + 1])

        for k in range(2):
            nc.sync.dma_start(p_d[k], p_fin[k])
        nc.sync.dma_start(se_d[:], se_fin[0:1, :])

    nc.compile()
    return nc


def _get_nc(s_core=S_CORE):
    if s_core not in _NC_CACHE:
        _NC_CACHE[s_core] = _build_nc(s_core)
    return _NC_CACHE[s_core]


def _host_smalls(h_t, W_h_w, W_h_b, W_w, W_b, beta_w):
    q = h_t[:, 0, :].astype(np.float64) @ W_h_w.T.astype(np.float64) \
        + W_h_b + W_b                                  # [b, beta]
    # per batch-half: qs[p, m*B_CORE+b] = q[bh*B_CORE+b, m*128+p]
    qs3 = q.T.reshape(4, 128, B).transpose(1, 0, 2)    # [128, 4, 64]
    qs_h = [np.ascontiguousarray(
        qs3[:, :, bh * B_CORE:(bh + 1) * B_CORE].reshape(128, 4 * B_CORE)
    ).astype(np.float32) for bh in range(N_BH)]
    wt = np.ascontiguousarray(
        W_w.T.reshape(2, 128, BETA).transpose(1, 0, 2).reshape(128, 2 * BETA)
    ).astype(BF16)
    bw = beta_w[0].astype(np.float32)
    bwr = np.ascontiguousarray(
        np.repeat(bw.reshape(4, 128).T[:, :, None], 128, axis=2).reshape(128, BETA)
    ).astype(BF16)
    return qs_h, wt, bwr


_PROFILE = False
_LAST_PERF = {}


def kernel(**inputs):
    from concourse.bass_utils import run_bass_kernel_spmd

    V = np.asarray(inputs["V"], dtype=np.float32)
    h_t = np.asarray(inputs["h_t"], dtype=np.float32)
    W_h_w = np.asarray(inputs["W_h_w"], dtype=np.float32)
    W_h_b = np.asarray(inputs["W_h_b"], dtype=np.float32)
    W_w = np.asarray(inputs["W_w"], dtype=np.float32)
    W_b = np.asarray(inputs["W_b"], dtype=np.float32)
    beta_w = np.asarray(inputs["beta_w"], dtype=np.float32)
    beta_b = np.asarray(inputs["beta_b"], dtype=np.float32)

    qs_h, wt, bwr = _host_smalls(h_t, W_h_w, W_h_b, W_w, W_b, beta_w)

    rows = HP // N_HPQ
    Vb = V.astype(BF16)
    in_maps = []
    core_meta = []
    for k in range(N_HPQ):
        Vq = Vb[k * rows:(k + 1) * rows].reshape(S_CORE, C_DIM, B)
        for bh in range(N_BH):
            # [s, c, b-half] -> [c, b, s] contiguous
            vk = np.ascontiguousarray(
                Vq[:, :, bh * B_CORE:(bh + 1) * B_CORE].transpose(1, 2, 0))
            in_maps.append({"v": vk, "wt": wt, "qs": qs_h[bh], "bwr": bwr})
            core_meta.append(bh)

    nc = _get_nc()
    res = run_bass_kernel_spmd(nc, in_maps, core_ids=list(range(NCORES)),
                               trace=_PROFILE)
    if _PROFILE:
        _LAST_PERF["exec_time_ns"] = res.exec_time_ns
        _LAST_PERF["trace"] = res.instructions_and_trace
    P = np.zeros((C_DIM, B), np.float64)
    SE = np.zeros((B,), np.float64)
    for bh, r in zip(core_meta, res.results):
        sl = slice(bh * B_CORE, (bh + 1) * B_CORE)
        P[:, sl] += r["p_out"].reshape(C_DIM, B_CORE)
        SE[sl] += r["se_out"][0]
    # softmax is shift-invariant so beta_b cancels; no max-sub needed (|E|<=~8)
    C = (P / SE).T.reshape(B, 1, C_DIM)
    return C.astype(np.float32)

